# revision 20
# baseline (speedup 1.0000x reference)
"""Multi-head attention (B=4, S=2048, D=1024, 16 heads x 64) on 8 NeuronCores.

Sharding: DP=4 over batch x TP=2 over heads (8 heads/core).
Each core computes, for one batch element and half the heads:
    qhT = (q @ Wq + bq)^T       [512, 2048]   (bf16, head-dim on partitions)
    khT = (k @ Wk + bk)^T       [512, 2048]
    vh  = v @ Wv                [2048, 520]   (natural layout, k on partitions,
                                               65-col per head: 64 dims + ones)
    per (head-pair, q-chunk): flash-style S^T = kh @ qh^T, p = exp(scale*S^T),
      PV with the 65-wide lhsT gives outT rows 0:64 AND the softmax
      denominator L in row 64 of the same accumulation (no separate L matmul).
      Normalize: rec = 1/L (DVE), partition-broadcast rec to 64 rows (GpSimd),
      outT = P * rec (DVE); the odd head's rows are DMA-shifted to
      partitions 64:128 of outT.
    partial_out = outT^T @ Wo_shard          [2048, 1024]  (fp32)
Host sums the TP pair partials and adds the bias terms (bv @ Wo + bo).

Inputs are transposed + cast to bf16 on the host so no on-device transposes
are needed (matmul contraction must be on the partition dim for both operands).
"""

import os
import sys

sys.path.insert(0, "/opt/trn_rl_repo")

import numpy as np
import ml_dtypes

S = 2048          # sequence length
DM = 1024         # model dim
HD = 512          # local head-dim total (8 heads x 64) per core (TP=2)
NB = 4            # batch
NCORES = 8
P = 128
DK = 64
HP65 = 65         # per-head lhsT width: 64 dims + ones column
VW = 8 * HP65     # 520
SCALE = 1.0 / 8.0  # 1/sqrt(64)

_CACHE = {}


def _build_nc():
    import concourse.bass as bass  # noqa: F401
    import concourse.mybir as mybir
    from concourse import bacc, tile
    from contextlib import ExitStack

    BF = mybir.dt.bfloat16
    F32 = mybir.dt.float32
    Exp = mybir.ActivationFunctionType.Exp

    nc = bacc.Bacc("TRN2", target_bir_lowering=False, debug=False, num_swdge_queues=4)

    qT = nc.dram_tensor("qT", [DM, S], BF, kind="ExternalInput")
    kT = nc.dram_tensor("kT", [DM, S], BF, kind="ExternalInput")
    vT = nc.dram_tensor("vT", [DM, S], BF, kind="ExternalInput")
    wq = nc.dram_tensor("wq", [DM, HD], BF, kind="ExternalInput")
    wk = nc.dram_tensor("wk", [DM, HD], BF, kind="ExternalInput")
    wv = nc.dram_tensor("wv", [DM, HD], BF, kind="ExternalInput")
    wo = nc.dram_tensor("wo", [HD, DM], BF, kind="ExternalInput")
    bq = nc.dram_tensor("bq", [HD], F32, kind="ExternalInput")
    bk = nc.dram_tensor("bk", [HD], F32, kind="ExternalInput")
    out = nc.dram_tensor("out", [S, DM], F32, kind="ExternalOutput")
    DEBUG = os.environ.get("KERNEL_DEBUG", "0") == "1"
    dbg = (
        nc.dram_tensor("dbg", [P, 2560], F32, kind="ExternalOutput")
        if DEBUG
        else None
    )

    NM = DM // P      # 8 m-chunks
    NHP = HD // P     # 4 head pairs
    NSC = S // 512    # 4 s-chunks of 512
    NJ = S // P       # 16 k-chunks

    with ExitStack() as ctx:
        tc = ctx.enter_context(tile.TileContext(nc))

        const = ctx.enter_context(tc.tile_pool(name="const", bufs=1))
        wq_pool = ctx.enter_context(tc.tile_pool(name="wq_pool", bufs=8))
        wk_pool = ctx.enter_context(tc.tile_pool(name="wk_pool", bufs=8))
        wv_pool = ctx.enter_context(tc.tile_pool(name="wv_pool", bufs=8))
        wo_pool = ctx.enter_context(tc.tile_pool(name="wo_pool", bufs=4))
        inpool = ctx.enter_context(tc.tile_pool(name="inpool", bufs=16))
        qh_pool = ctx.enter_context(tc.tile_pool(name="qh_pool", bufs=4))
        kh_pool = ctx.enter_context(tc.tile_pool(name="kh_pool", bufs=4))
        vh_pool = ctx.enter_context(tc.tile_pool(name="vh_pool", bufs=16))
        outT_pool = ctx.enter_context(tc.tile_pool(name="outT_pool", bufs=4))
        p_pool = ctx.enter_context(tc.tile_pool(name="p_pool", bufs=6))
        rec_pool = ctx.enter_context(tc.tile_pool(name="rec_pool", bufs=2))
        recb_pool = ctx.enter_context(tc.tile_pool(name="recb_pool", bufs=2))
        tmp_pool = ctx.enter_context(tc.tile_pool(name="tmp_pool", bufs=2))
        stage_pool = ctx.enter_context(tc.tile_pool(name="stage_pool", bufs=3))
        if DEBUG:
            dbg_pool = ctx.enter_context(tc.tile_pool(name="dbg_pool", bufs=1))
        st_ps = ctx.enter_context(tc.tile_pool(name="st_ps", bufs=3, space="PSUM"))
        pv_ps = ctx.enter_context(tc.tile_pool(name="pv_ps", bufs=2, space="PSUM"))

        # constants
        # selector for the reciprocal broadcast: only row 64 is ones, so a
        # standard K=128 matmul replicates rec_bf row 64 into 64 partitions
        sel_row = const.tile([P, DK], BF, tag="sel")
        nc.vector.memset(sel_row[:], 0.0)
        nc.vector.memset(sel_row[DK : DK + 1, :], 1.0)
        # persistent zeroed tiles holding the reciprocal row (row 64); all
        # other partitions stay 0.0 forever so the K=128 bcast matmul sees
        # clean zeros (not NaN garbage) on the unused partitions
        recbf_tiles = [
            rec_pool.tile([P, 1024], BF, tag="recbf", name=f"recbf{i}")
            for i in range(2)
        ]
        for t in recbf_tiles:
            nc.vector.memset(t[:], 0.0)
        bq_sb = const.tile([P, NHP], F32, tag="bq")
        nc.gpsimd.dma_start(bq_sb[:], bq[:].rearrange("(f p) -> p f", p=P))
        bk_sb = const.tile([P, NHP], F32, tag="bk")
        nc.gpsimd.dma_start(bk_sb[:], bk[:].rearrange("(f p) -> p f", p=P))

        def load_weight(pool, handle, tag, eng=None):
            eng = eng or nc.sync
            tiles = []
            for m in range(NM):
                t = pool.tile([P, HD], BF, tag=tag)
                eng.dma_start(t[:], handle[m * P : (m + 1) * P, :])
                tiles.append(t)
            return tiles

        def load_input(handle, tag, eng=None):
            eng = eng or nc.sync
            tiles = []
            for m in range(NM):
                t = inpool.tile([P, S], BF, tag="in")
                eng.dma_start(t[:], handle[m * P : (m + 1) * P, :])
                tiles.append(t)
            return tiles

        # ---- v projection chains (vh[s, hd] natural layout with a ones
        # column interleaved per head: [64 dims | 1] x 8 heads = 520 cols);
        # first 8 run upfront, the rest ride as fillers in block (0,0) ----
        wv_sb = load_weight(wv_pool, wv, "wv")
        vT_sb = load_input(vT, "vT")
        vh_sb = [vh_pool.tile([P, VW], BF, tag="vh", name=f"vh{i}") for i in range(NJ)]
        for i in range(NJ):
            # ones columns (col 64 of each head's 65-col group)
            nc.vector.memset(
                vh_sb[i][:].rearrange("p (h c) -> p h c", h=8)[:, :, DK : DK + 1], 1.0
            )

        def vproj_chain_ops(sc):
            cell = {}

            def mk(m):
                def op():
                    if m == 0:
                        if sc % 3 == 2:
                            cell["ps"] = pv_ps.tile([P, HD], F32, tag="pvps", name="vps2")
                        else:
                            cell["ps"] = st_ps.tile([P, 1024], F32, tag="stps", name="vps")[:, 0:HD]
                    nc.tensor.matmul(
                        cell["ps"][:],
                        lhsT=vT_sb[m][:, sc * P : (sc + 1) * P],
                        rhs=wv_sb[m][:],
                        start=(m == 0),
                        stop=(m == NM - 1),
                    )
                return op

            ops = [mk(m) for m in range(NM)]

            def ev():
                nc.vector.tensor_copy(
                    vh_sb[sc][:].rearrange("p (h c) -> p h c", h=8)[:, :, 0:DK],
                    cell["ps"][:].rearrange("p (h c) -> p h c", h=8),
                )

            ops.append(ev)
            return ops

        # ---- q/k projections: qhT/khT [hd, s], head-pair-major tiles ----
        wk_sb = load_weight(wk_pool, wk, "wk", eng=nc.gpsimd)
        kT_sb = load_input(kT, "kT")
        wq_sb = load_weight(wq_pool, wq, "wq", eng=nc.gpsimd)
        qT_sb = load_input(qT, "qT")

        qhT_sb = [qh_pool.tile([P, S], BF, tag="qh", name=f"qhT{i}") for i in range(NHP)]
        khT_sb = [kh_pool.tile([P, S], BF, tag="kh", name=f"khT{i}") for i in range(NHP)]
        outT_sb = [outT_pool.tile([P, S], BF, tag="outT", name=f"outT{i}") for i in range(NHP)]

        def proj_chain_ops(w_sb, x_sb, dst, bias_sb, hp, sc):
            """One projection output chunk as a list of single-op closures."""
            cell = {}

            def mk(m):
                def op():
                    if m == 0:
                        cell["ps"] = st_ps.tile([P, 1024], F32, tag="stps", name="fps")[:, 0:512]
                    nc.tensor.matmul(
                        cell["ps"][:],
                        lhsT=w_sb[m][:, hp * P : (hp + 1) * P],
                        rhs=x_sb[m][:, sc * 512 : (sc + 1) * 512],
                        start=(m == 0),
                        stop=(m == NM - 1),
                    )
                return op

            ops = [mk(m) for m in range(NM)]

            def ev():
                nc.vector.tensor_scalar_add(
                    dst[:, sc * 512 : (sc + 1) * 512],
                    cell["ps"][:],
                    bias_sb[:, hp : hp + 1],
                )

            ops.append(ev)
            return ops

        def fc_chain_ops(sc, ec):
            ss = slice(sc * P, (sc + 1) * P)
            es = slice(ec * 512, (ec + 1) * 512)
            cell = {}

            def mk(hp):
                def op():
                    if hp == 0:
                        cell["ps"] = st_ps.tile([P, 1024], F32, tag="stps", name="fps")[:, 0:512]
                    nc.tensor.matmul(
                        cell["ps"][:],
                        lhsT=outT_sb[hp][:, ss],
                        rhs=wo_sb[hp][:, es],
                        start=(hp == 0),
                        stop=(hp == NHP - 1),
                    )
                return op

            ops = [mk(hp) for hp in range(NHP)]

            def ev():
                stg = stage_pool.tile([P, 512], F32, tag="stg", name="stg")
                nc.vector.tensor_copy(stg[:], cell["ps"][:])
                nc.gpsimd.dma_start(out[ss, es], stg[:])

            ops.append(ev)
            return ops

        from collections import deque

        fillers = deque()

        def drain(n):
            for _ in range(n):
                if not fillers:
                    return
                fillers.popleft()()

        # upfront (inside the input-DMA window): v-proj, kp0 and qp0 chunk 0
        # interleaved round-robin over 5 psum slots so slot-recycle latency
        # amortizes — just enough for block (0,0) to start
        up = [vproj_chain_ops(sc) for sc in range(NJ)]
        for sc in range(NSC):
            up.insert(5 * sc + 4, proj_chain_ops(wk_sb, kT_sb, khT_sb[0], bk_sb, 0, sc))
        up.append(proj_chain_ops(wq_sb, qT_sb, qhT_sb[0], bq_sb, 0, 0))
        for ops in up:
            for op in ops:
                op()
        wo_sb = []
        for hp in range(NHP):
            t = wo_pool.tile([P, DM], BF, tag="wo")
            nc.gpsimd.dma_start(t[:], wo[hp * P : (hp + 1) * P, :])
            wo_sb.append(t)

        # remaining projection work rides along inside the attention blocks
        for sc in range(1, NSC):
            fillers.extend(proj_chain_ops(wq_sb, qT_sb, qhT_sb[0], bq_sb, 0, sc))
        for nhp in range(1, NHP):
            for sc in range(NSC):
                fillers.extend(proj_chain_ops(wk_sb, kT_sb, khT_sb[nhp], bk_sb, nhp, sc))
            for sc in range(NSC):
                fillers.extend(proj_chain_ops(wq_sb, qT_sb, qhT_sb[nhp], bq_sb, nhp, sc))

        # ---- attention, flash style, software-pipelined emission so the PE
        # queue keeps st(j+3) ahead of PV(j); block tails (last PVs + the
        # normalize chain) spill into the next block's emission. ----
        carry = []  # deferred ops from the previous block

        def attn_block(hp, qc, budget):
            qs = slice(qc * 512, (qc + 1) * 512)
            he, ho = 2 * hp, 2 * hp + 1
            state = {}
            p_tiles = {}

            def ensure_pv_tiles():
                if "P0" not in state:
                    state["P0"] = pv_ps.tile([P, 512], F32, tag="pvps", name="P0ps")
                    state["P1"] = pv_ps.tile([P, 512], F32, tag="pvps", name="P1ps")

            def emit_st(j):
                ks = slice(j * P, (j + 1) * P)
                st = st_ps.tile([P, 1024], F32, tag="stps")
                nc.tensor.matmul(
                    st[:, 0:512],
                    lhsT=khT_sb[hp][0:64, ks],
                    rhs=qhT_sb[hp][0:64, qs],
                    start=True,
                    stop=True,
                    tile_position=(0, 0),
                )
                nc.tensor.matmul(
                    st[:, 512:1024],
                    lhsT=khT_sb[hp][64:128, ks],
                    rhs=qhT_sb[hp][64:128, qs],
                    start=True,
                    stop=True,
                    tile_position=(64, 0),
                )
                p = p_pool.tile([P, 1024], BF, tag="p")
                nc.scalar.activation(p[:], st[:], Exp, scale=SCALE)
                p_tiles[j] = p

            def emit_pv(j):
                ensure_pv_tiles()
                P0, P1 = state["P0"], state["P1"]
                p = p_tiles.pop(j)
                first, last = (j == 0), (j == NJ - 1)
                nc.tensor.matmul(
                    P0[0:HP65, :],
                    lhsT=vh_sb[j][:, he * HP65 : (he + 1) * HP65],
                    rhs=p[:, 0:512],
                    start=first,
                    stop=last,
                    skip_group_check=True,
                )
                nc.tensor.matmul(
                    P1[0:HP65, :],
                    lhsT=vh_sb[j][:, ho * HP65 : (ho + 1) * HP65],
                    rhs=p[:, 512:1024],
                    start=first,
                    stop=last,
                    skip_group_check=True,
                )

            LAG = 3
            for j in range(NJ):
                emit_st(j)
                for _ in range(4):
                    if carry:
                        carry.pop(0)()
                if j >= LAG:
                    emit_pv(j - LAG)
                if not carry:
                    drain(budget)

            def mk_pv(j):
                return lambda: emit_pv(j)

            def mk_norm():
                cellN = {}

                def recips():
                    # full-width [0:65] so the custom-DVE op starts at
                    # partition 0 (base-partition-64 slices misbehave);
                    # rows 0:64 compute junk reciprocals that are never read
                    rec = rec_pool.tile([P, 1024], F32, tag="rec")
                    cellN["rec"] = rec
                    nc.vector.reciprocal_approx_fast(
                        rec[0:HP65, 0:512], state["P0"][0:HP65, :]
                    )
                    nc.vector.reciprocal_approx_fast(
                        rec[0:HP65, 512:1024], state["P1"][0:HP65, :]
                    )

                def to_bf():
                    rec_bf = recbf_tiles[(hp * NSC + qc) % 2]
                    cellN["rec_bf"] = rec_bf
                    nc.vector.tensor_copy(
                        rec_bf[DK : DK + 1, :], cellN["rec"][DK : DK + 1, :]
                    )

                def bcast_mm():
                    # replicate the reciprocal row (partition 64) to the
                    # first 64 partitions via a standard K=128 matmul with a
                    # one-hot selector (only row 64 of sel_row is nonzero)
                    recb = st_ps.tile([P, 1024], F32, tag="stps", name="recb")
                    cellN["recb_ps"] = recb
                    nc.tensor.matmul(
                        recb[0:DK, 0:512],
                        lhsT=sel_row[:],
                        rhs=cellN["rec_bf"][:, 0:512],
                        start=True,
                        stop=True,
                    )
                    nc.tensor.matmul(
                        recb[0:DK, 512:1024],
                        lhsT=sel_row[:],
                        rhs=cellN["rec_bf"][:, 512:1024],
                        start=True,
                        stop=True,
                    )

                def evac_recb():
                    recb_sb = recb_pool.tile([P, 1024], BF, tag="recb")
                    cellN["recb"] = recb_sb
                    nc.vector.tensor_copy(
                        recb_sb[0:DK, :], cellN["recb_ps"][0:DK, :]
                    )

                def mul_e():
                    nc.vector.tensor_mul(
                        outT_sb[hp][0:DK, qs],
                        state["P0"][0:DK, :],
                        cellN["recb"][0:DK, 0:512],
                    )

                def mul_o():
                    tmp = tmp_pool.tile([P, 512], BF, tag="tmp")
                    cellN["tmp"] = tmp
                    nc.vector.tensor_mul(
                        tmp[0:DK, :],
                        state["P1"][0:DK, :],
                        cellN["recb"][0:DK, 512:1024],
                    )

                def shift():
                    nc.sync.dma_start(outT_sb[hp][DK:P, qs], cellN["tmp"][0:DK, :])

                def dump():
                    stg1 = dbg_pool.tile([P, 512], F32, tag="dbg1", name="dbg1")
                    nc.vector.tensor_copy(stg1[:], state["P0"][:])
                    nc.sync.dma_start(dbg[:, 0:512], stg1[:])
                    stg2 = dbg_pool.tile([P, 512], F32, tag="dbg2", name="dbg2")
                    nc.vector.tensor_copy(stg2[:], state["P1"][:])
                    nc.sync.dma_start(dbg[:, 512:1024], stg2[:])
                    stg3 = dbg_pool.tile([P, 1024], F32, tag="dbg3", name="dbg3")
                    nc.vector.tensor_copy(stg3[:], cellN["recb"][:])
                    nc.sync.dma_start(dbg[:, 1024:2048], stg3[:])
                    stg4 = dbg_pool.tile([P, 512], F32, tag="dbg4", name="dbg4")
                    nc.vector.tensor_copy(stg4[:], outT_sb[hp][:, 0:512])
                    nc.sync.dma_start(dbg[:, 2048:2560], stg4[:])

                ops = [recips, to_bf, bcast_mm, evac_recb, mul_e, mul_o, shift]
                if DEBUG and hp == 0 and qc == 0:
                    ops.append(dump)
                return ops

            return [mk_pv(j) for j in range(NJ - LAG, NJ)] + mk_norm()

        for hp in range(NHP):
            for qc in range(NSC):
                carry = attn_block(hp, qc, 3 if hp == NHP - 1 else 2)
                if hp == NHP - 1:
                    for sc in range(qc * 4, qc * 4 + 4):
                        fillers.extend(fc_chain_ops(sc, 0))
                        fillers.extend(fc_chain_ops(sc, 1))
        for op in carry:
            op()

        # whatever is left (last fc chunks)
        while fillers:
            fillers.popleft()()

    nc.compile()
    return nc


def _get_nc():
    if "nc" not in _CACHE:
        _CACHE["nc"] = _build_nc()
    return _CACHE["nc"]


def kernel(q, k, v, Wq, bq, Wk, bk, Wv, bv, Wo, bo):
    from concourse.bass_utils import run_bass_kernel_spmd

    bf16 = ml_dtypes.bfloat16
    q, k, v = (np.asarray(x, np.float32) for x in (q, k, v))
    Wq, bq, Wk, bk, Wv, bv, Wo, bo = (
        np.asarray(x, np.float32) for x in (Wq, bq, Wk, bk, Wv, bv, Wo, bo)
    )

    in_maps = []
    for c in range(NCORES):
        b, t = c // 2, c % 2
        hs = slice(t * HD, (t + 1) * HD)
        in_maps.append(
            {
                "qT": q[b].T.astype(bf16),
                "kT": k[b].T.astype(bf16),
                "vT": v[b].T.astype(bf16),
                "wq": Wq[:, hs].astype(bf16),
                "wk": Wk[:, hs].astype(bf16),
                "wv": Wv[:, hs].astype(bf16),
                "wo": Wo[hs, :].astype(bf16),
                "bq": np.ascontiguousarray(bq[hs]),
                "bk": np.ascontiguousarray(bk[hs]),
            }
        )

    nc = _get_nc()
    trace = os.environ.get("KERNEL_TRACE", "0") == "1"
    res = run_bass_kernel_spmd(
        nc, in_maps, core_ids=list(range(NCORES)), trace=trace
    )
    if trace:
        print(f"HW exec time: {res.exec_time_ns} ns")

    host_bias = (bv @ Wo + bo).astype(np.float32)
    full = np.empty((NB, S, DM), np.float32)
    for b in range(NB):
        full[b] = res.results[2 * b]["out"] + res.results[2 * b + 1]["out"] + host_bias
    return full


# revision 27
# speedup vs baseline: 1.0759x; 1.0759x over previous
"""Multi-head attention (B=4, S=2048, D=1024, 16 heads x 64) on 8 NeuronCores.

Sharding: DP=4 over batch x TP=2 over heads (8 heads/core).
Each core computes, for one batch element and half the heads:
    qhT = (q @ Wq + bq)^T       [512, 2048]   (bf16, head-dim on partitions)
    khT = (k @ Wk + bk)^T       [512, 2048]
    vh  = v @ Wv                [2048, 520]   (natural layout, k on partitions,
                                               65-col per head: 64 dims + ones)
    per (head-pair, q-chunk): flash-style S^T = kh @ qh^T, p = exp(scale*S^T),
      PV with the 65-wide lhsT gives outT rows 0:64 AND the softmax
      denominator L in row 64 of the same accumulation (no separate L matmul).
      Normalize: rec = 1/L (DVE), partition-broadcast rec to 64 rows (GpSimd),
      outT = P * rec (DVE); the odd head's rows are DMA-shifted to
      partitions 64:128 of outT.
    partial_out = outT^T @ Wo_shard          [2048, 1024]  (fp32)
Host sums the TP pair partials and adds the bias terms (bv @ Wo + bo).

Inputs are transposed + cast to bf16 on the host so no on-device transposes
are needed (matmul contraction must be on the partition dim for both operands).
"""

import os
import sys

sys.path.insert(0, "/opt/trn_rl_repo")

import numpy as np
import ml_dtypes

S = 2048          # sequence length
DM = 1024         # model dim
HD = 512          # local head-dim total (8 heads x 64) per core (TP=2)
NB = 4            # batch
NCORES = 8
P = 128
DK = 64
HP65 = 65         # per-head lhsT width: 64 dims + ones column
VW = 8 * HP65     # 520
SCALE = 1.0 / 8.0  # 1/sqrt(64)

_CACHE = {}


def _build_nc():
    import concourse.bass as bass  # noqa: F401
    import concourse.mybir as mybir
    from concourse import bacc, tile
    from contextlib import ExitStack

    BF = mybir.dt.bfloat16
    F32 = mybir.dt.float32
    Exp = mybir.ActivationFunctionType.Exp

    nc = bacc.Bacc("TRN2", target_bir_lowering=False, debug=False, num_swdge_queues=4)

    qT = nc.dram_tensor("qT", [DM, S], BF, kind="ExternalInput")
    kT = nc.dram_tensor("kT", [DM, S], BF, kind="ExternalInput")
    vT = nc.dram_tensor("vT", [DM, S], BF, kind="ExternalInput")
    wq = nc.dram_tensor("wq", [DM, HD], BF, kind="ExternalInput")
    wk = nc.dram_tensor("wk", [DM, HD], BF, kind="ExternalInput")
    wv = nc.dram_tensor("wv", [DM, HD], BF, kind="ExternalInput")
    wo = nc.dram_tensor("wo", [HD, DM], BF, kind="ExternalInput")
    bq = nc.dram_tensor("bq", [HD], F32, kind="ExternalInput")
    bk = nc.dram_tensor("bk", [HD], F32, kind="ExternalInput")
    out = nc.dram_tensor("out", [S, DM], F32, kind="ExternalOutput")
    DEBUG = os.environ.get("KERNEL_DEBUG", "0") == "1"
    dbg = (
        nc.dram_tensor("dbg", [P, 2560], F32, kind="ExternalOutput")
        if DEBUG
        else None
    )

    NM = DM // P      # 8 m-chunks
    NHP = HD // P     # 4 head pairs
    NSC = S // 512    # 4 s-chunks of 512
    NJ = S // P       # 16 k-chunks

    with ExitStack() as ctx:
        tc = ctx.enter_context(tile.TileContext(nc))

        const = ctx.enter_context(tc.tile_pool(name="const", bufs=1))
        wq_pool = ctx.enter_context(tc.tile_pool(name="wq_pool", bufs=8))
        wk_pool = ctx.enter_context(tc.tile_pool(name="wk_pool", bufs=8))
        wv_pool = ctx.enter_context(tc.tile_pool(name="wv_pool", bufs=8))
        wo_pool = ctx.enter_context(tc.tile_pool(name="wo_pool", bufs=4))
        inpool = ctx.enter_context(tc.tile_pool(name="inpool", bufs=16))
        qh_pool = ctx.enter_context(tc.tile_pool(name="qh_pool", bufs=4))
        kh_pool = ctx.enter_context(tc.tile_pool(name="kh_pool", bufs=4))
        vh_pool = ctx.enter_context(tc.tile_pool(name="vh_pool", bufs=16))
        outT_pool = ctx.enter_context(tc.tile_pool(name="outT_pool", bufs=4))
        p_pool = ctx.enter_context(tc.tile_pool(name="p_pool", bufs=6))
        rec_pool = ctx.enter_context(tc.tile_pool(name="rec_pool", bufs=2))
        recb_pool = ctx.enter_context(tc.tile_pool(name="recb_pool", bufs=2))
        tmp_pool = ctx.enter_context(tc.tile_pool(name="tmp_pool", bufs=2))
        stage_pool = ctx.enter_context(tc.tile_pool(name="stage_pool", bufs=3))
        if DEBUG:
            dbg_pool = ctx.enter_context(tc.tile_pool(name="dbg_pool", bufs=1))
        st_ps = ctx.enter_context(tc.tile_pool(name="st_ps", bufs=3, space="PSUM"))
        pv_ps = ctx.enter_context(tc.tile_pool(name="pv_ps", bufs=2, space="PSUM"))

        # constants
        # selector for the reciprocal broadcast: only row 64 is ones, so a
        # standard K=128 matmul replicates rec_bf row 64 into 64 partitions
        sel_row = const.tile([P, DK], BF, tag="sel")
        nc.vector.memset(sel_row[:], 0.0)
        nc.vector.memset(sel_row[DK : DK + 1, :], 1.0)
        # persistent zeroed tiles holding the reciprocal row (row 64); all
        # other partitions stay 0.0 forever so the K=128 bcast matmul sees
        # clean zeros (not NaN garbage) on the unused partitions
        recbf_tiles = [
            rec_pool.tile([P, 1024], BF, tag="recbf", name=f"recbf{i}")
            for i in range(2)
        ]
        for t in recbf_tiles:
            nc.vector.memset(t[:], 0.0)
        bq_sb = const.tile([P, NHP], F32, tag="bq")
        nc.gpsimd.dma_start(bq_sb[:], bq[:].rearrange("(f p) -> p f", p=P))
        bk_sb = const.tile([P, NHP], F32, tag="bk")
        nc.gpsimd.dma_start(bk_sb[:], bk[:].rearrange("(f p) -> p f", p=P))

        def load_weight(pool, handle, tag, eng=None):
            eng = eng or nc.sync
            tiles = []
            for m in range(NM):
                t = pool.tile([P, HD], BF, tag=tag)
                eng.dma_start(t[:], handle[m * P : (m + 1) * P, :])
                tiles.append(t)
            return tiles

        def load_input(handle, tag, eng=None):
            eng = eng or nc.sync
            tiles = []
            for m in range(NM):
                t = inpool.tile([P, S], BF, tag="in")
                eng.dma_start(t[:], handle[m * P : (m + 1) * P, :])
                tiles.append(t)
            return tiles

        # ---- v projection chains (vh[s, hd] natural layout with a ones
        # column interleaved per head: [64 dims | 1] x 8 heads = 520 cols);
        # first 8 run upfront, the rest ride as fillers in block (0,0) ----
        wv_sb = load_weight(wv_pool, wv, "wv")
        vT_sb = load_input(vT, "vT")
        vh_sb = [vh_pool.tile([P, VW], BF, tag="vh", name=f"vh{i}") for i in range(NJ)]
        for i in range(NJ):
            # ones columns (col 64 of each head's 65-col group)
            nc.vector.memset(
                vh_sb[i][:].rearrange("p (h c) -> p h c", h=8)[:, :, DK : DK + 1], 1.0
            )

        def vproj_chain_ops(sc):
            cell = {}

            def mk(m):
                def op():
                    if m == 0:
                        if sc % 3 == 2:
                            cell["ps"] = pv_ps.tile([P, HD], F32, tag="pvps", name="vps2")
                        else:
                            cell["ps"] = st_ps.tile([P, 1024], F32, tag="stps", name="vps")[:, 0:HD]
                    nc.tensor.matmul(
                        cell["ps"][:],
                        lhsT=vT_sb[m][:, sc * P : (sc + 1) * P],
                        rhs=wv_sb[m][:],
                        start=(m == 0),
                        stop=(m == NM - 1),
                    )
                return op

            ops = [mk(m) for m in range(NM)]

            def ev():
                nc.vector.tensor_copy(
                    vh_sb[sc][:].rearrange("p (h c) -> p h c", h=8)[:, :, 0:DK],
                    cell["ps"][:].rearrange("p (h c) -> p h c", h=8),
                )

            ops.append(ev)
            return ops

        # ---- q/k projections: qhT/khT [hd, s], head-pair-major tiles ----
        wk_sb = load_weight(wk_pool, wk, "wk", eng=nc.gpsimd)
        kT_sb = load_input(kT, "kT")
        wq_sb = load_weight(wq_pool, wq, "wq", eng=nc.gpsimd)
        qT_sb = load_input(qT, "qT")

        qhT_sb = [qh_pool.tile([P, S], BF, tag="qh", name=f"qhT{i}") for i in range(NHP)]
        khT_sb = [kh_pool.tile([P, S], BF, tag="kh", name=f"khT{i}") for i in range(NHP)]
        outT_sb = [outT_pool.tile([P, S], BF, tag="outT", name=f"outT{i}") for i in range(NHP)]

        def proj_chain_ops(w_sb, x_sb, dst, bias_sb, hp, sc):
            """One projection output chunk as a list of single-op closures."""
            cell = {}

            def mk(m):
                def op():
                    if m == 0:
                        cell["ps"] = st_ps.tile([P, 1024], F32, tag="stps", name="fps")[:, 0:512]
                    nc.tensor.matmul(
                        cell["ps"][:],
                        lhsT=w_sb[m][:, hp * P : (hp + 1) * P],
                        rhs=x_sb[m][:, sc * 512 : (sc + 1) * 512],
                        start=(m == 0),
                        stop=(m == NM - 1),
                    )
                return op

            ops = [mk(m) for m in range(NM)]

            def ev():
                nc.vector.tensor_scalar_add(
                    dst[:, sc * 512 : (sc + 1) * 512],
                    cell["ps"][:],
                    bias_sb[:, hp : hp + 1],
                )

            ops.append(ev)
            return ops

        def fc_chain_ops(sc, ec):
            ss = slice(sc * P, (sc + 1) * P)
            es = slice(ec * 512, (ec + 1) * 512)
            cell = {}

            def mk(hp):
                def op():
                    if hp == 0:
                        cell["ps"] = st_ps.tile([P, 1024], F32, tag="stps", name="fps")[:, 0:512]
                    nc.tensor.matmul(
                        cell["ps"][:],
                        lhsT=outT_sb[hp][:, ss],
                        rhs=wo_sb[hp][:, es],
                        start=(hp == 0),
                        stop=(hp == NHP - 1),
                    )
                return op

            ops = [mk(hp) for hp in range(NHP)]

            def ev():
                stg = stage_pool.tile([P, 512], F32, tag="stg", name="stg")
                nc.vector.tensor_copy(stg[:], cell["ps"][:])
                nc.gpsimd.dma_start(out[ss, es], stg[:])

            ops.append(ev)
            return ops

        from collections import deque

        fillers = deque()

        def drain(n):
            for _ in range(n):
                if not fillers:
                    return
                fillers.popleft()()

        # upfront (inside the input-DMA window): v-proj, kp0 and qp0 chunk 0
        # interleaved round-robin over 5 psum slots so slot-recycle latency
        # amortizes — just enough for block (0,0) to start
        up = [vproj_chain_ops(sc) for sc in range(NJ)]
        for sc in range(NSC):
            up.insert(5 * sc + 4, proj_chain_ops(wk_sb, kT_sb, khT_sb[0], bk_sb, 0, sc))
        up.append(proj_chain_ops(wq_sb, qT_sb, qhT_sb[0], bq_sb, 0, 0))
        for ops in up:
            for op in ops:
                op()
        wo_sb = []
        for hp in range(NHP):
            t = wo_pool.tile([P, DM], BF, tag="wo")
            nc.gpsimd.dma_start(t[:], wo[hp * P : (hp + 1) * P, :])
            wo_sb.append(t)

        # remaining projection work rides along inside the attention blocks
        for sc in range(1, NSC):
            fillers.extend(proj_chain_ops(wq_sb, qT_sb, qhT_sb[0], bq_sb, 0, sc))
        for nhp in range(1, NHP):
            for sc in range(NSC):
                fillers.extend(proj_chain_ops(wk_sb, kT_sb, khT_sb[nhp], bk_sb, nhp, sc))
            for sc in range(NSC):
                fillers.extend(proj_chain_ops(wq_sb, qT_sb, qhT_sb[nhp], bq_sb, nhp, sc))

        # ---- attention, flash style, software-pipelined emission so the PE
        # queue keeps st(j+3) ahead of PV(j); block tails (last PVs + the
        # normalize chain) spill into the next block's emission. ----
        carry = []  # deferred ops from the previous block

        def attn_block(hp, qc, budget):
            qs = slice(qc * 512, (qc + 1) * 512)
            he, ho = 2 * hp, 2 * hp + 1
            state = {}
            p_tiles = {}

            def ensure_pv_tiles():
                if "P0" not in state:
                    state["P0"] = pv_ps.tile([P, 512], F32, tag="pvps", name="P0ps")
                    state["P1"] = pv_ps.tile([P, 512], F32, tag="pvps", name="P1ps")

            def emit_st(j):
                ks = slice(j * P, (j + 1) * P)
                st = st_ps.tile([P, 1024], F32, tag="stps")
                nc.tensor.matmul(
                    st[:, 0:512],
                    lhsT=khT_sb[hp][0:64, ks],
                    rhs=qhT_sb[hp][0:64, qs],
                    start=True,
                    stop=True,
                    tile_position=(0, 0),
                )
                nc.tensor.matmul(
                    st[:, 512:1024],
                    lhsT=khT_sb[hp][64:128, ks],
                    rhs=qhT_sb[hp][64:128, qs],
                    start=True,
                    stop=True,
                    tile_position=(64, 0),
                )
                p = p_pool.tile([P, 1024], BF, tag="p")
                nc.scalar.activation(p[:], st[:], Exp, scale=SCALE)
                p_tiles[j] = p

            def emit_pv(j):
                ensure_pv_tiles()
                P0, P1 = state["P0"], state["P1"]
                p = p_tiles.pop(j)
                first, last = (j == 0), (j == NJ - 1)
                nc.tensor.matmul(
                    P0[0:HP65, :],
                    lhsT=vh_sb[j][:, he * HP65 : (he + 1) * HP65],
                    rhs=p[:, 0:512],
                    start=first,
                    stop=last,
                    skip_group_check=True,
                )
                nc.tensor.matmul(
                    P1[0:HP65, :],
                    lhsT=vh_sb[j][:, ho * HP65 : (ho + 1) * HP65],
                    rhs=p[:, 512:1024],
                    start=first,
                    stop=last,
                    skip_group_check=True,
                )

            LAG = 3
            for j in range(NJ):
                emit_st(j)
                for _ in range(2):
                    if carry:
                        op = carry.pop(0)
                        if op is not None:
                            op()
                if j >= LAG:
                    emit_pv(j - LAG)
                if not carry:
                    drain(budget)

            def mk_pv(j):
                return lambda: emit_pv(j)

            def mk_norm():
                cellN = {}

                def evac_p0():
                    # evacuate the PV accumulators to SBUF right away so the
                    # PSUM slots are free before the next block's PV(0)
                    # (start=True clears the bank — reads must come first)
                    E0 = tmp_pool.tile([P, 512], F32, tag="E0", name="E0")
                    cellN["E0"] = E0
                    nc.vector.tensor_copy(E0[0:HP65, :], state["P0"][0:HP65, :])

                def evac_p1():
                    E1 = tmp_pool.tile([P, 512], F32, tag="E1", name="E1")
                    cellN["E1"] = E1
                    nc.vector.tensor_copy(E1[0:HP65, :], state["P1"][0:HP65, :])

                def recips():
                    # full-width [0:65] so the custom-DVE op starts at
                    # partition 0 (base-partition-64 slices misbehave);
                    # rows 0:64 compute junk reciprocals that are never read
                    rec = rec_pool.tile([P, 1024], F32, tag="rec")
                    cellN["rec"] = rec
                    nc.vector.reciprocal_approx_fast(
                        rec[0:HP65, 0:512], cellN["E0"][0:HP65, :]
                    )
                    nc.vector.reciprocal_approx_fast(
                        rec[0:HP65, 512:1024], cellN["E1"][0:HP65, :]
                    )

                def to_bf():
                    rec_bf = recbf_tiles[(hp * NSC + qc) % 2]
                    cellN["rec_bf"] = rec_bf
                    nc.vector.tensor_copy(
                        rec_bf[DK : DK + 1, :], cellN["rec"][DK : DK + 1, :]
                    )

                def bcast_mm():
                    # replicate the reciprocal row (partition 64) to the
                    # first 64 partitions via a standard K=128 matmul with a
                    # one-hot selector (only row 64 of sel_row is nonzero)
                    recb = st_ps.tile([P, 1024], F32, tag="stps", name="recb")
                    cellN["recb_ps"] = recb
                    nc.tensor.matmul(
                        recb[0:DK, 0:512],
                        lhsT=sel_row[:],
                        rhs=cellN["rec_bf"][:, 0:512],
                        start=True,
                        stop=True,
                    )
                    nc.tensor.matmul(
                        recb[0:DK, 512:1024],
                        lhsT=sel_row[:],
                        rhs=cellN["rec_bf"][:, 512:1024],
                        start=True,
                        stop=True,
                    )

                def evac_recb():
                    recb_sb = recb_pool.tile([P, 1024], BF, tag="recb")
                    cellN["recb"] = recb_sb
                    nc.vector.tensor_copy(
                        recb_sb[0:DK, :], cellN["recb_ps"][0:DK, :]
                    )

                def mul_e():
                    nc.vector.tensor_mul(
                        outT_sb[hp][0:DK, qs],
                        cellN["E0"][0:DK, :],
                        cellN["recb"][0:DK, 0:512],
                    )

                def mul_o():
                    tmp = tmp_pool.tile([P, 512], BF, tag="tmp")
                    cellN["tmp"] = tmp
                    nc.vector.tensor_mul(
                        tmp[0:DK, :],
                        cellN["E1"][0:DK, :],
                        cellN["recb"][0:DK, 512:1024],
                    )

                def shift():
                    nc.sync.dma_start(outT_sb[hp][DK:P, qs], cellN["tmp"][0:DK, :])

                def dump():
                    stg1 = dbg_pool.tile([P, 512], F32, tag="dbg1", name="dbg1")
                    nc.vector.tensor_copy(stg1[:], cellN["E0"][:])
                    nc.sync.dma_start(dbg[:, 0:512], stg1[:])
                    stg2 = dbg_pool.tile([P, 512], F32, tag="dbg2", name="dbg2")
                    nc.vector.tensor_copy(stg2[:], cellN["E1"][:])
                    nc.sync.dma_start(dbg[:, 512:1024], stg2[:])
                    stg3 = dbg_pool.tile([P, 1024], F32, tag="dbg3", name="dbg3")
                    nc.vector.tensor_copy(stg3[:], cellN["recb"][:])
                    nc.sync.dma_start(dbg[:, 1024:2048], stg3[:])
                    stg4 = dbg_pool.tile([P, 512], F32, tag="dbg4", name="dbg4")
                    nc.vector.tensor_copy(stg4[:], outT_sb[hp][:, 0:512])
                    nc.sync.dma_start(dbg[:, 2048:2560], stg4[:])

                # spacers (None) pace the chain so the bcast matmul reaches
                # the PE FIFO only after its DVE inputs are ready — the
                # dependency chain never stalls the PE at block boundaries
                ops = [
                    evac_p0,
                    evac_p1,
                    recips,
                    None,
                    to_bf,
                    None,
                    None,
                    bcast_mm,
                    evac_recb,
                    mul_e,
                    mul_o,
                    shift,
                ]
                if DEBUG and hp == 0 and qc == 0:
                    ops.append(dump)
                return ops

            return [mk_pv(j) for j in range(NJ - LAG, NJ)] + mk_norm()

        for hp in range(NHP):
            for qc in range(NSC):
                carry = attn_block(hp, qc, 3 if hp == NHP - 1 else 2)
                if hp == NHP - 1:
                    for sc in range(qc * 4, qc * 4 + 4):
                        fillers.extend(fc_chain_ops(sc, 0))
                        fillers.extend(fc_chain_ops(sc, 1))
        for op in carry:
            if op is not None:
                op()

        # whatever is left (last fc chunks)
        while fillers:
            fillers.popleft()()

    nc.compile()
    return nc


def _get_nc():
    if "nc" not in _CACHE:
        _CACHE["nc"] = _build_nc()
    return _CACHE["nc"]


def kernel(q, k, v, Wq, bq, Wk, bk, Wv, bv, Wo, bo):
    from concourse.bass_utils import run_bass_kernel_spmd

    bf16 = ml_dtypes.bfloat16
    q, k, v = (np.asarray(x, np.float32) for x in (q, k, v))
    Wq, bq, Wk, bk, Wv, bv, Wo, bo = (
        np.asarray(x, np.float32) for x in (Wq, bq, Wk, bk, Wv, bv, Wo, bo)
    )

    in_maps = []
    for c in range(NCORES):
        b, t = c // 2, c % 2
        hs = slice(t * HD, (t + 1) * HD)
        in_maps.append(
            {
                "qT": q[b].T.astype(bf16),
                "kT": k[b].T.astype(bf16),
                "vT": v[b].T.astype(bf16),
                "wq": Wq[:, hs].astype(bf16),
                "wk": Wk[:, hs].astype(bf16),
                "wv": Wv[:, hs].astype(bf16),
                "wo": Wo[hs, :].astype(bf16),
                "bq": np.ascontiguousarray(bq[hs]),
                "bk": np.ascontiguousarray(bk[hs]),
            }
        )

    nc = _get_nc()
    trace = os.environ.get("KERNEL_TRACE", "0") == "1"
    res = run_bass_kernel_spmd(
        nc, in_maps, core_ids=list(range(NCORES)), trace=trace
    )
    if trace:
        print(f"HW exec time: {res.exec_time_ns} ns")

    host_bias = (bv @ Wo + bo).astype(np.float32)
    full = np.empty((NB, S, DM), np.float32)
    for b in range(NB):
        full[b] = res.results[2 * b]["out"] + res.results[2 * b + 1]["out"] + host_bias
    return full
